# revision 40
# baseline (speedup 1.0000x reference)
"""Multi-head causal attention (B=2, C=2048, E=1024, H=16) on 8 NeuronCores.

Sharding: tensor-parallel over (batch, head-group): core = b*4 + g handles
batch b and heads [4g, 4g+4). Each core computes Q^T/K^T/V projections for
its 4 heads, causal attention, and its partial output projection
ctx_slice @ Wo_slice -> [2048, 1024]. Host sums the 4 partials per batch
(the tensor-parallel all-reduce, done at unshard time) and adds bo.

Dataflow is fully transposed so no on-device transposes are needed:
  Q^T = Wq_s.T @ x^T          [256 f, 2048 t]   (f = head-local features)
  K^T = Wk_s.T @ x^T          [256 f, 2048 t]
  V   = x    @ Wv_s           [2048 t, 256 f]  (natural layout, + ones col)
  s^T = K^T_h.T @ Q^T_h       [128 k, 512 q] per (head, k-chunk, q-tile)
  P^T = exp((s^T + M) / 32)   (M = -1e7 additive causal mask on the
                               diagonal 128x128 block, added in PSUM)
  ctx_aug^T = V_aug.T @ P^T   [65, 512], row 64 = softmax normalizer l
  ctx^T = ctx_aug^T[0:64] * (1/l)  (broadcast via gpsimd)
  out_partial = ctx^T.T @ Wo_s     [2048, 1024] fp32

All matmul operands are fp16 (PE upconverts to FP22, accumulates fp32 in
PSUM). fp8 projections were tried and rejected: e4m3 quantization of
x/Wq/Wk/Wv alone costs ~2.4-3.5e-2 max rel err (softmax averaging does
not wash out the tails), over the 2e-2 gate. Projection matmuls are
emitted as deferred pieces drained between attention chunks so the PE
queue always holds runnable work (a >3us PE-idle gap drops HAM to 4/8
duty and halves the matmul rate).
"""
import numpy as np

import concourse.bass as bass
import concourse.tile as tile
from concourse import bacc, mybir
from concourse.bass_utils import run_bass_kernel_spmd

F16 = mybir.dt.float16
F32 = mybir.dt.float32

B, C, E, H = 2, 2048, 1024, 16
NH = 4              # heads per core
D = 64              # head dim
FS = NH * D         # 256 features per core
EC = E // 128       # 8 e-chunks
QT = 512            # q tile size
NQ = C // QT        # 4 q tiles
KC = C // 128       # 16 k chunks
TC = C // 128       # 16 token chunks
# module scales by sqrt(E)
SCALE = 1.0 / np.sqrt(np.float32(E))
MASK_NEG = -1.0e7   # additive causal mask (-1e7 * SCALE << -80: exp -> 0)

_CACHED_NC = None


def build():
    nc = bacc.Bacc("TRN2", target_bir_lowering=False, debug=False, num_devices=8)
    xT = nc.dram_tensor("xT", [E, C], F16, kind="ExternalInput")
    # weights arrive pre-laid-out for contiguous DMA:
    # wq/wk/wv: [128, EC, FS]; wo: [128, 2, E]
    wq = nc.dram_tensor("wq", [128, EC, FS], F16, kind="ExternalInput")
    wk = nc.dram_tensor("wk", [128, EC, FS], F16, kind="ExternalInput")
    wv = nc.dram_tensor("wv", [128, EC, FS], F16, kind="ExternalInput")
    wo = nc.dram_tensor("wo", [128, 2, E], F16, kind="ExternalInput")
    msk = nc.dram_tensor("msk", [128, 128], F32, kind="ExternalInput")
    out = nc.dram_tensor("out", [E, C], F16, kind="ExternalOutput")  # out^T

    with tile.TileContext(nc) as tc:
        with tc.tile_pool(name="const", bufs=1) as cp, \
             tc.tile_pool(name="work", bufs=1) as wp, \
             tc.tile_pool(name="ps", bufs=1, space="PSUM") as ps:
            # ---- resident SBUF tensors ----
            xT_sb = cp.tile([128, EC, C], F16)
            wq_sb = cp.tile([128, EC, FS], F16)
            wk_sb = cp.tile([128, EC, FS], F16)
            wv_sb = cp.tile([128, EC, FS], F16)
            wo_sb = cp.tile([128, 2, E], F16)
            msk_sb = cp.tile([128, 128], F32)
            qt_sb = cp.tile([128, 2, C], F16)
            kt_sb = cp.tile([128, 2, C], F16)
            v_sb = cp.tile([128, TC, NH * (D + 1)], F16)   # +1: ones col per head
            ctxt_sb = cp.tile([128, 2, C], F16)

            # ---- PE warm-up emitted first: runs during the input-DMA
            # window so HAM un-throttles before the real stream starts
            wu = wp.tile([128, 512], F16, tag="wu", bufs=1)
            nc.vector.memset(wu[:], 0.5)
            ones_sb = wp.tile([128, 64], F16, tag="ones", bufs=1)
            nc.vector.memset(ones_sb[:], 1.0)
            wups = ps.tile([128, 512], F32, tag="ctx", bufs=2, name="wups")
            for i in range(24):
                nc.tensor.matmul(wups[:], lhsT=wu[:, 0:128], rhs=wu[:],
                                 start=True, stop=True)

            # ones cols of V (only these bytes; the proj copy fills the rest)
            nc.vector.memset(
                v_sb[:].rearrange("p t (h x) -> p t h x", h=NH)[:, :, :, D:D + 1],
                1.0)

            # ---- input DMAs, ordered to match first-consumption order
            nc.sync.dma_start(wk_sb[:], wk[:])
            for c in range(EC):
                nc.sync.dma_start(xT_sb[:, c, :], xT[c * 128:(c + 1) * 128, :])
            nc.sync.dma_start(wq_sb[:], wq[:])
            nc.sync.dma_start(wv_sb[:], wv[:])
            nc.sync.dma_start(msk_sb[:], msk[:])
            nc.sync.dma_start(wo_sb[:], wo[:])

            # ---- projection building blocks ----
            def proj_k_leadin(w_sb, o_sb, g2):
                # chunk-outer across ALL FOUR j-tiles: every x chunk is
                # consumed by 4 matmuls the moment its DMA lands, so the
                # K projection finishes ~1 chunk after the x DMA does.
                # j0/j1 accumulate in the pj psum slots; j2/j3 borrow the
                # ctx slots (idle until the first attention).
                pps = [ps.tile([128, QT], F32, tag="pj", bufs=2,
                               name=f"pk_{o_sb.name}_{g2}_{jj}")
                       for jj in range(2)]
                pps += [ps.tile([128, QT], F32, tag="ctx", bufs=2,
                                name=f"pk_{o_sb.name}_{g2}_{2 + jj}")
                        for jj in range(2)]
                wufill = ps.tile([128, 2 * QT], F32, tag="big", bufs=2,
                                 name="wufill")
                for c in range(EC):
                    for j in range(NQ):
                        nc.tensor.matmul(
                            pps[j][:],
                            lhsT=w_sb[:, c, 128 * g2:128 * (g2 + 1)],
                            rhs=xT_sb[:, c, QT * j:QT * (j + 1)],
                            start=(c == 0), stop=(c == EC - 1),
                        )
                    # the 4 matmuls drain ~0.6us faster than the next x
                    # chunk's DMA lands — fill the hole so HAM stays 8/8
                    for _ in range(2):
                        nc.tensor.matmul(wufill[:, 0:QT], lhsT=wu[:, 0:128],
                                         rhs=wu[:], start=True, stop=True)
                for j in range(NQ):
                    nc.vector.tensor_copy(
                        o_sb[:, g2, QT * j:QT * (j + 1)], pps[j][:])

            def proj_kq_allj(w_sb, o_sb, g2, defer=False):
                # j-pair structured (pj slots only — safe inside attention)
                for j0 in (0, 2):
                    cell = {}

                    def mk(c, j0=j0):
                        def f():
                            if c == 0:
                                cell["pps"] = [
                                    ps.tile([128, QT], F32, tag="pj", bufs=2,
                                            name=f"pk_{o_sb.name}_{g2}_{j0 + jj}")
                                    for jj in range(2)]
                            for jj in range(2):
                                j = j0 + jj
                                nc.tensor.matmul(
                                    cell["pps"][jj][:],
                                    lhsT=w_sb[:, c, 128 * g2:128 * (g2 + 1)],
                                    rhs=xT_sb[:, c, QT * j:QT * (j + 1)],
                                    start=(c == 0), stop=(c == EC - 1),
                                )
                        return f

                    def cp(j0=j0):
                        def f():
                            for jj in range(2):
                                nc.vector.tensor_copy(
                                    o_sb[:, g2,
                                         QT * (j0 + jj):QT * (j0 + jj + 1)],
                                    cell["pps"][jj][:])
                        return f

                    pieces = [mk(c) for c in range(EC)] + [cp()]
                    if defer:
                        pend.extend(pieces)
                    else:
                        for p in pieces:
                            p()

            # pending projection pieces: one matmul (or epilogue copy) per
            # closure, drained into the attention conveyor so the PE queue
            # always holds runnable work between potentially-stalling
            # score/ctx matmuls (a >3us PE gap drops HAM to 4/8 duty).
            pend = []

            def drain(n=None):
                k = len(pend) if n is None else min(n, len(pend))
                for _ in range(k):
                    pend.pop(0)()

            def proj_kq(w_sb, o_sb, g2, j, defer=False):
                cell = {}

                def mm(c):
                    def f():
                        if c == 0:
                            cell["pp"] = ps.tile(
                                [128, QT], F32, tag="pj", bufs=2,
                                name=f"pp_{o_sb.name}_{g2}_{j}")
                        nc.tensor.matmul(
                            cell["pp"][:],
                            lhsT=w_sb[:, c, 128 * g2:128 * (g2 + 1)],
                            rhs=xT_sb[:, c, QT * j:QT * (j + 1)],
                            start=(c == 0), stop=(c == EC - 1),
                        )
                    return f

                pieces = [mm(c) for c in range(EC)]
                pieces.append(lambda: nc.vector.tensor_copy(
                    o_sb[:, g2, QT * j:QT * (j + 1)], cell["pp"][:]))
                if defer:
                    pend.extend(pieces)
                else:
                    for p in pieces:
                        p()

            def proj_v(t, defer=False):
                cell = {}

                def mm(c):
                    def f():
                        if c == 0:
                            cell["pp"] = ps.tile(
                                [128, FS], F32, tag="pj", bufs=2,
                                name=f"pp_v_{t}")
                        nc.tensor.matmul(
                            cell["pp"][:],
                            lhsT=xT_sb[:, c, 128 * t:128 * (t + 1)],
                            rhs=wv_sb[:, c, :],
                            start=(c == 0), stop=(c == EC - 1),
                        )
                    return f

                pieces = [mm(c) for c in range(EC)]
                pieces.append(lambda: nc.vector.tensor_copy(
                    v_sb[:, t, :].rearrange("p (h x) -> p h x", h=NH)[:, :, 0:D],
                    cell["pp"][:].rearrange("p (h d) -> p h d", h=NH)))
                if defer:
                    pend.extend(pieces)
                else:
                    for p in pieces:
                        p()

            # ---- attention: head pairs (0,1)/(2,3); both heads' score tiles
            # ---- share one [128, 2*QT] psum so exp is a single wide op
            def emit_scores(heads, j, c):
                """s^T pair -> (diag: +M in psum) -> one exp -> fp16 P^T.

                Diagonal chunks (c = 4j + r): queries q < 128r are fully
                masked, so all work is restricted to q in [128r, QT); the
                128x128 diagonal block gets the additive -1e7 mask before
                exp, which zeroes its upper triangle."""
                q0 = 128 * (c - 4 * j) if c >= 4 * j else 0
                st = ps.tile([128, 2 * QT], F32, tag="big", bufs=2,
                             name=f"st_{heads[0]}_{j}_{c}")
                for i, h in enumerate(heads):
                    g2, po = h // 2, 64 * (h % 2)
                    nc.tensor.matmul(
                        st[:, QT * i + q0:QT * (i + 1)],
                        lhsT=kt_sb[po:po + 64, g2, 128 * c:128 * (c + 1)],
                        rhs=qt_sb[po:po + 64, g2, QT * j + q0:QT * (j + 1)],
                        start=True, stop=True,
                    )
                st3 = st[:].rearrange("p (b q) -> p b q", b=2)
                if c >= 4 * j:  # diagonal: additive causal mask in psum
                    nc.vector.tensor_add(
                        st3[:, :, q0:q0 + 128], st3[:, :, q0:q0 + 128],
                        msk_sb[:].unsqueeze(1).broadcast_to([128, 2, 128]))
                pt = wp.tile([128, 2 * QT], F16, tag="pt", bufs=6)
                nc.scalar.activation(
                    pt[:].rearrange("p (b q) -> p b q", b=2)[:, :, q0:QT],
                    st3[:, :, q0:QT],
                    mybir.ActivationFunctionType.Exp, scale=SCALE)
                return pt

            def attention(pair, j, last=False, pre_norm=None):
                heads = (2 * pair, 2 * pair + 1)
                nk = 4 * (j + 1)   # causal: k chunks 0..nk-1
                ctx_ps = {h: ps.tile([128, QT], F32, tag="ctx", bufs=2,
                                     name=f"ctx_{pair}_{j}_{h}")
                          for h in heads}
                # 2-chunk bursts: the PE queue alternates one 64-contract
                # block (2 concurrent score pairs) with one 128-contract
                # block (4 ctx matmuls + drained proj pieces), halving the
                # tiling-mode switches (each switch drains the PE, ~80ns).
                # drain() sits between scores and ctx so deferred pieces'
                # epilogue copies are queued before ctx matmuls that might
                # depend on them (e.g. V tiles for this attention).
                pts = {}
                for c in (0, 1):                # software-pipeline prologue
                    if c < nk:
                        pts[c] = emit_scores(heads, j, c)
                # pace the pending pieces across all bursts so late
                # (ACT-bound) bursts of big-nk attentions keep PE filler
                dn = max(1, -(-2 * len(pend) // nk))
                for c0 in range(0, nk, 2):
                    for c in (c0 + 2, c0 + 3):
                        if c < nk:
                            pts[c] = emit_scores(heads, j, c)
                    drain(dn)  # keep independent proj work in the PE queue
                    for c in (c0, c0 + 1):
                        pt = pts.pop(c)
                        q0 = 128 * (c - 4 * j) if c >= 4 * j else 0
                        for i, h in enumerate(heads):
                            nc.tensor.matmul(
                                ctx_ps[h][0:D + 1, q0:QT],
                                lhsT=v_sb[:, c, (D + 1) * h:(D + 1) * (h + 1)],
                                rhs=pt[:, QT * i + q0:QT * (i + 1)],
                                start=(c == 0), stop=(c == nk - 1),
                            )
                # normalize: ctx^T[d, q] * (1/l[q]).  Stage both heads' psum
                # to SBUF first (frees the ctx psum slots fast so following
                # matmuls aren't gated on the slow recip chain).
                stgs = {}
                for i, h in enumerate(heads):
                    stg = wp.tile([D + 1, QT], F16, tag="stg", bufs=4,
                                  name=f"stg_{pair}_{j}_{h}")
                    nc.vector.tensor_copy(stg[:], ctx_ps[h][0:D + 1, :])
                    stgs[h] = stg
                if pre_norm is not None:
                    # independent PE work emitted here fills the queue while
                    # the normalize chain runs (keeps HAM at 8/8 in the tail)
                    pre_norm[0]()
                rc = wp.tile([64, 2 * QT], F32, tag="rc", bufs=4)
                if last:
                    # broadcast l via a 1-contraction PE matmul (the PE is
                    # idle here anyway; the gpsimd broadcast costs 1.8us)
                    bcp = ps.tile([64, 2 * QT], F32, tag="big", bufs=2,
                                  name="bc_last")
                    for i, h in enumerate(heads):
                        nc.tensor.matmul(
                            bcp[:, QT * i:QT * (i + 1)],
                            lhsT=ones_sb[D:D + 1, :], rhs=stgs[h][D:D + 1, :],
                            start=True, stop=True)
                    if pre_norm is not None:
                        pre_norm[1]()
                    nc.vector.reciprocal_approx_fast(rc[:], bcp[:])
                else:
                    lr = wp.tile([1, 2 * QT], F32, tag="lr", bufs=4)
                    for i, h in enumerate(heads):
                        nc.vector.tensor_copy(lr[:, QT * i:QT * (i + 1)],
                                              stgs[h][D:D + 1, :])
                    bc = wp.tile([64, 2 * QT], F32, tag="bc", bufs=4)
                    nc.gpsimd.partition_broadcast(bc[:], lr[:])
                    nc.vector.reciprocal_approx_fast(rc[:], bc[:])
                for i, h in enumerate(heads):
                    g2, po = h // 2, 64 * (h % 2)
                    nc.vector.tensor_mul(
                        ctxt_sb[po:po + 64, g2, QT * j:QT * (j + 1)],
                        stgs[h][0:D, :], rc[:, QT * i:QT * (i + 1)])

            def wo_sweep(tt, defer=False):
                # partial out^T[e, tt-slice] = Wo_s.T @ ctx^T for q-tile tt.
                # Deferred pieces drain inside pair-1 attention so the wo
                # matmuls and output DMA overlap compute instead of piling
                # up in the kernel tail.
                def piece(ec):
                    def f():
                        pp = ps.tile([128, QT], F32, tag="pj", bufs=2,
                                     name=f"pp_wo_{ec}_{tt}")
                        for g2 in range(2):
                            nc.tensor.matmul(
                                pp[:],
                                lhsT=wo_sb[:, g2, 128 * ec:128 * (ec + 1)],
                                rhs=ctxt_sb[:, g2, QT * tt:QT * (tt + 1)],
                                start=(g2 == 0), stop=(g2 == 1),
                            )
                        ot = wp.tile([128, QT], F16, tag="ot", bufs=4)
                        if ec % 2 == 0:
                            nc.vector.tensor_copy(ot[:], pp[:])
                        else:
                            nc.scalar.activation(
                                ot[:], pp[:],
                                mybir.ActivationFunctionType.Copy)
                        nc.sync.dma_start(
                            out[128 * ec:128 * (ec + 1),
                                QT * tt:QT * (tt + 1)], ot[:])
                    return f

                pieces = [piece(ec) for ec in range(EC)]
                if defer:
                    pend.extend(pieces)
                else:
                    for p in pieces:
                        p()

            proj_k_leadin(wk_sb, kt_sb, 0)
            proj_kq(wq_sb, qt_sb, 0, 0)
            for t in range(3):
                proj_v(t)
            proj_v(3, defer=True)   # ctx chunk 3 comes late enough
            for j in range(NQ):
                if j < NQ - 1:
                    proj_kq(wq_sb, qt_sb, 0, j + 1, defer=True)
                    for t in range(4 * (j + 1), 4 * (j + 2)):
                        proj_v(t, defer=True)
                else:
                    proj_kq_allj(wk_sb, kt_sb, 1, defer=True)
                    proj_kq(wq_sb, qt_sb, 1, 0, defer=True)
                # last=True: the pair transition is exposed to this
                # attention's normalize chain — use the PE-broadcast path
                attention(0, j, last=(j == NQ - 1))
                drain()
            for j in range(NQ):
                if j < NQ - 1:
                    proj_kq(wq_sb, qt_sb, 1, j + 1, defer=True)
                if j >= 1:
                    wo_sweep(j - 1, defer=True)
                attention(1, j, last=(j == NQ - 1))
                drain()
            wo_sweep(3)
    nc.compile()
    return nc


def _causal_mask():
    # [128 k, 128 q] additive mask for the diagonal block: 0 keep, -1e7 drop
    k = np.arange(128)[:, None]
    q = np.arange(128)[None, :]
    return np.where(k <= q, 0.0, MASK_NEG).astype(np.float32)


def _w_in(w):
    # [E, FS] -> [128 p, EC chunks, FS] (e = c*128 + p)
    return np.ascontiguousarray(
        w.reshape(EC, 128, FS).transpose(1, 0, 2)).astype(np.float16)


def _wo_in(w):
    # [FS, E] -> [128 p, 2 g, E] (f = g*128 + p)
    return np.ascontiguousarray(
        w.reshape(2, 128, E).transpose(1, 0, 2)).astype(np.float16)


def _in_maps(x, Wq, Wk, Wv, Wo):
    msk = _causal_mask()
    in_maps = []
    for b in range(B):
        xT_h = np.ascontiguousarray(x[b].T).astype(np.float16)
        for g in range(4):
            s = slice(g * FS, (g + 1) * FS)
            in_maps.append({
                "xT": xT_h,
                "wq": _w_in(Wq[:, s]),
                "wk": _w_in(Wk[:, s]),
                "wv": _w_in(Wv[:, s]),
                "wo": _wo_in(np.ascontiguousarray(Wo[s, :])),
                "msk": msk,
            })
    return in_maps


def kernel(x, Wq, Wk, Wv, Wo, bo):
    global _CACHED_NC
    x = np.asarray(x, np.float32)
    Wq = np.asarray(Wq, np.float32)
    Wk = np.asarray(Wk, np.float32)
    Wv = np.asarray(Wv, np.float32)
    Wo = np.asarray(Wo, np.float32)
    bo = np.asarray(bo, np.float32)

    if _CACHED_NC is None:
        _CACHED_NC = build()
    nc = _CACHED_NC

    res = run_bass_kernel_spmd(nc, _in_maps(x, Wq, Wk, Wv, Wo),
                               core_ids=list(range(8)))

    out = np.empty((B, C, E), np.float32)
    for b in range(B):
        acc = res.results[b * 4 + 0]["out"].astype(np.float32)
        for g in range(1, 4):
            acc += res.results[b * 4 + g]["out"]
        out[b] = acc.T + bo          # kernel emits out^T
    return out
